# revision 53
# baseline (speedup 1.0000x reference)
"""Causal multi-head attention (B=4, T=2048, C=1024, 16 heads) on 8 TRN2 NeuronCores.

Sharding: data-parallel over (batch, q-chunk-pair). Core 2*b+h handles batch b
and two 512-row q-chunks chosen so every core runs an identical program:
  core (b,0): chunk A = rows [0:512]     (program kv extent 1024)
              chunk B = rows [1536:2048] (program kv extent 2048)
  core (b,1): chunk A = rows [512:1024]  (kv extent 1024)
              chunk B = rows [1024:1536] (kv extent 2048, data extent 1536)
Causality inside the rectangles is enforced with per-core {0,1} multiplicative
masks supplied as data, so the instruction stream is core-independent (SPMD).

Everything on-device lives transposed ([feature, token]): softmax denominators
come out of the TensorEngine via a ones-column appended to V, and no on-device
transposes are needed; the host transposes x in and the output out.

Inputs/weights/activations are bf16 (PE at full rate, fp32 PSUM accumulation);
the normalization path is fp32. Score matmuls for a head pair run on PE
row-groups 0-63 / 64-127 concurrently (contract dim is 64).

The K-projection bias is dropped: s[kv,q] += q_vec(q)@bk is constant along
the softmax (kv) axis, so it cancels in softmax exactly.

All four weight matrices are SBUF-resident (wq/wk/wo cycle a 2-slot pool;
wv is resident as two halves), so projection matmuls never wait on DMA.

Softmax epilogue: the two PSUM ctx accumulators are evacuated immediately
with plain copies (frees the banks for the next head pair); the denominator
row then takes a deferred off-critical-path chain: DMA hop to partition 0 ->
reciprocal_approx_fast (~51 ULP, 5x faster than exact) -> GpSimd partition
broadcast -> DVE multiply.

The emission order interleaves PE-heavy projection work into the ACT-bound
attention phases: K/V projections for kv [1024:2048] and the chunk-B Q
projection are spread between chunk-A head pairs; the chunk-A output
projection is spread between chunk-B head pairs.
"""

import numpy as np
import ml_dtypes

B, T, C, NH, D = 4, 2048, 1024, 16, 64
P = 128
CH = 512                # q-chunk size
KV_EXT = (1024, 2048)   # program kv extent for chunk A / chunk B

_CACHE = {}


def _build():
    import concourse.bacc as bacc
    import concourse.tile as tile
    import concourse.mybir as mybir
    from concourse.bass import ts, ds

    f32 = mybir.dt.float32
    bf16 = mybir.dt.bfloat16
    ID = mybir.ActivationFunctionType.Identity
    EXP = mybir.ActivationFunctionType.Exp
    COPY = mybir.ActivationFunctionType.Copy
    MUL = mybir.AluOpType.mult
    ADD = mybir.AluOpType.add

    nc = bacc.Bacc("TRN2", target_bir_lowering=False, debug=False, num_devices=8)

    def din(name, shape, dt=bf16):
        return nc.dram_tensor(name, list(shape), dt, kind="ExternalInput").ap()

    xqT = din("xqT", (C, 2 * CH))    # x^T, this core's q rows (A then B)
    xkvT = din("xkvT", (C, T))       # x^T, full batch (for K/V)
    wqT = din("wqT", (C, C))         # (Wq/8)^T
    wkT = din("wkT", (C, C))
    wvT = din("wvT", (C, C))
    woT = din("woT", (C, C))
    bq = din("bq", (P, C // P), f32)     # bq/8, chunked [128, 8]
    bo = din("bo", (P, C // P), f32)
    maskA = din("maskA", (KV_EXT[0], CH))     # {0,1}, [kv, q] chunk A
    maskB = din("maskB", (1024, CH))          # chunk B, kv in [1024:2048]
    out = nc.dram_tensor("out", [C, 2 * CH], f32, kind="ExternalOutput").ap()

    KC = C // P        # 8 contraction chunks
    NT = T // P        # 16 kv chunks of the full batch

    wq_v = wqT.rearrange("(ko p) m -> p ko m", p=P)
    wk_v = wkT.rearrange("(ko p) m -> p ko m", p=P)
    wo_v = woT.rearrange("(ko p) m -> p ko m", p=P)
    wv_v = wvT.rearrange("(ko p) c -> p ko c", p=P)
    xkv_v = xkvT.rearrange("(ko p) t -> p ko t", p=P)
    xq_v = xqT.rearrange("(ko p) t -> p ko t", p=P)
    maskA_v = maskA.rearrange("(ko p) q -> p ko q", p=P)
    maskB_v = maskB.rearrange("(ko p) q -> p ko q", p=P)

    from contextlib import ExitStack
    with ExitStack() as ctx:
        tc = ctx.enter_context(tile.TileContext(nc))

        consts = ctx.enter_context(tc.tile_pool(name="consts", bufs=1))
        big = ctx.enter_context(tc.tile_pool(name="big", bufs=1))
        wres = ctx.enter_context(tc.tile_pool(name="wres", bufs=2))
        xkpool = ctx.enter_context(tc.tile_pool(name="xk", bufs=2))
        xvpool = ctx.enter_context(tc.tile_pool(name="xv", bufs=2))
        qpool = ctx.enter_context(tc.tile_pool(name="q", bufs=1))
        mpool = ctx.enter_context(tc.tile_pool(name="m", bufs=1))
        xqpool = ctx.enter_context(tc.tile_pool(name="xq", bufs=1))
        ptpool = ctx.enter_context(tc.tile_pool(name="pt", bufs=3))
        ctxpool = ctx.enter_context(tc.tile_pool(name="ctx", bufs=1))
        l0pool = ctx.enter_context(tc.tile_pool(name="l0", bufs=1))
        lbpool = ctx.enter_context(tc.tile_pool(name="lb", bufs=1))
        cspool = ctx.enter_context(tc.tile_pool(name="cs", bufs=3))
        cs2pool = ctx.enter_context(tc.tile_pool(name="cs2", bufs=2))
        opool = ctx.enter_context(tc.tile_pool(name="o", bufs=2))
        psumP = ctx.enter_context(tc.tile_pool(name="psumP", bufs=2, space="PSUM"))
        psumS = ctx.enter_context(tc.tile_pool(name="psumS", bufs=2, space="PSUM"))
        psumX = ctx.enter_context(tc.tile_pool(name="psumX", bufs=2, space="PSUM"))

        bq_sb = consts.tile([P, KC], f32)
        bo_sb = consts.tile([P, KC], f32)
        ones_t = consts.tile([P, D], f32)
        nc.vector.memset(ones_t[:], 1.0)

        # K^T split: one tile for kv [0:1024] plus 8 per-m tiles for kv
        # [1024:2048].  Dependencies are tile-granular, so the per-m split
        # lets chunk-B pair hp depend only on the kt-hi projection of m=hp,
        # which can then run interleaved inside the (ACT-bound) chunk-B
        # windows instead of inflating the PE-bound chunk-A windows.
        KT_lo = big.tile([P, KC, T // 2], bf16, name="KT_lo")
        kthi = [big.tile([P, T // 2], bf16, name=f"kthi{m}") for m in range(KC)]
        V_sb = big.tile([P, NT, NH, D + 1], bf16)   # V + ones col per chunk/head
        nc.vector.memset(V_sb[:, :, :, D : D + 1], 1.0)
        # Wv^T resident as 4 quarter-tiles: dependencies are tile-granular,
        # so the first v_proj matmul only waits for a 512KB transfer.
        wvt = [[big.tile([P, KC // 2, CH], bf16, name=f"wvt{c}{h}")
                for h in range(2)] for c in range(2)]

        # ---------- emission helpers ----------
        XK = {}

        def kt_proj(wk_sb, ft, m0, m1):
            """KT[:, m0:m1, 512*ft:...] from a cached xk tile. No bias: the
            K bias shifts scores by a q-only constant, softmax-invariant."""
            if ft not in XK:
                XK[ft] = xkpool.tile([P, KC, 512], bf16, tag="xk",
                                     name=f"xk{ft}")
                nc.sync.dma_start(XK[ft][:], xkv_v[:, :, ds(512 * ft, 512)])
            xk = XK[ft]
            for m in range(m0, m1):
                ps = psumP.tile([P, 512], f32, tag="psP", name=f"pk{ft}{m}")
                for k in range(KC):
                    nc.tensor.matmul(ps[:], wk_sb[:, k, ts(m, P)], xk[:, k, :],
                                     start=(k == 0), stop=(k == KC - 1))
                if ft < 2:
                    nc.scalar.activation(KT_lo[:, m, ds(512 * ft, 512)],
                                         ps[:], COPY)
                else:
                    nc.scalar.activation(kthi[m][:, ds(512 * (ft - 2), 512)],
                                         ps[:], COPY)

        XV = {}

        def xv_fetch(i):
            XV[i] = xvpool.tile([P, KC, P], bf16, tag="xv", name=f"xv{i}")
            nc.sync.dma_start(XV[i][:], xkv_v[:, :, ts(i, P)])

        def v_proj(i):
            """V rows [128*i : 128*(i+1)], all channels."""
            if i not in XV:
                xv_fetch(i)
            xv = XV.pop(i)
            for chh in range(2):
                ps = psumP.tile([P, 512], f32, tag="psP", name=f"pv{i}{chh}")
                for k in range(KC):
                    nc.tensor.matmul(ps[:], xv[:, k, :],
                                     wvt[chh][k // 4][:, k % 4, :],
                                     start=(k == 0), stop=(k == KC - 1))
                nc.scalar.activation(
                    V_sb[:, i, ds(8 * chh, 8), 0:D],
                    ps.rearrange("p (h d) -> p h d", d=D), COPY)

        QT = {}

        def q_proj(wq_sb, qc, m0, m1):
            if qc not in QT:
                QT[qc] = qpool.tile([P, KC, CH], bf16, tag=f"qt{qc}",
                                    name=f"qt{qc}")
            if ("xq", qc) not in QT:
                QT[("xq", qc)] = xqpool.tile([P, KC, CH], bf16, tag="xq",
                                             name=f"xq{qc}")
                nc.sync.dma_start(QT[("xq", qc)][:],
                                  xq_v[:, :, ds(CH * qc, CH)])
            xq = QT[("xq", qc)]
            for m in range(m0, m1):
                ps = psumP.tile([P, CH], f32, tag="psP", name=f"pq{qc}{m}")
                for k in range(KC):
                    nc.tensor.matmul(ps[:], wq_sb[:, k, ts(m, P)], xq[:, k, :],
                                     start=(k == 0), stop=(k == KC - 1))
                nc.scalar.activation(QT[qc][:, m, :], ps[:], ID,
                                     bias=bq_sb[:, m : m + 1])

        def o_proj(wo_sb, qc, rhs_of, m, dve_epi=False, pool=None):
            ps = (pool or psumP).tile([P, CH], f32,
                                      tag="psP" if pool is None else "psX",
                                      name=f"po{qc}{m}")
            for k in range(KC):
                nc.tensor.matmul(ps[:], wo_sb[:, k, ts(m, P)], rhs_of(k),
                                 start=(k == 0), stop=(k == KC - 1))
            o_sb = opool.tile([P, CH], f32, tag="o", name=f"o{qc}{m}")
            if dve_epi:
                # chunk-B windows are ACT(exp)-bound: bias-add there on DVE
                nc.vector.tensor_tensor(
                    o_sb[:], ps[:],
                    bo_sb[:, m : m + 1].to_broadcast((P, CH)), ADD)
            else:
                nc.scalar.activation(o_sb[:], ps[:], ID,
                                     bias=bo_sb[:, m : m + 1])
            nc.sync.dma_start(out[ts(m, P), ds(CH * qc, CH)], o_sb[:])

        def attn_pair(qc, hp, msk, ctxT, col, fast_tail=False):
            E = KV_EXT[qc]
            NKV = E // P
            ctx_ps = [psumX.tile([P, CH], f32, tag="psX", name=f"psX{qc}{hp}{i}")
                      for i in range(2)]
            for kvc in range(NKV):
                st = psumS.tile([P, 2, CH], f32, tag="psS",
                                name=f"psS{qc}{hp}{kvc}")
                for hh in range(2):
                    # contract dim 64 at PE row-group 64*hh: the two heads'
                    # score matmuls run concurrently in the array
                    if kvc < 8:
                        ktap = KT_lo[ds(64 * hh, 64), hp, ds(P * kvc, P)]
                    else:
                        ktap = kthi[hp][ds(64 * hh, 64), ds(P * (kvc - 8), P)]
                    nc.tensor.matmul(
                        st[:, hh, :], ktap,
                        QT[qc][ds(64 * hh, 64), hp, :],
                        start=True, stop=True)
                pt = ptpool.tile([P, 2, CH], bf16, tag="pt",
                                 name=f"pt{qc}{hp}{kvc}")
                nc.scalar.activation(pt[:], st[:], EXP)
                mi = kvc if qc == 0 else kvc - NKV // 2
                if mi >= 0:   # causal mask (chunk A: all; chunk B: kv >= 1024)
                    nc.vector.tensor_tensor(
                        pt[:], pt[:],
                        msk[:, mi : mi + 1, :].to_broadcast((P, 2, CH)), MUL)
                for hh in range(2):
                    nc.tensor.matmul(
                        ctx_ps[hh][0 : D + 1, :],
                        V_sb[:, kvc, 2 * hp + hh, :],
                        pt[:, hh, :],
                        start=(kvc == 0), stop=(kvc == NKV - 1))
            # Epilogue. Evacuate both PSUM banks immediately with plain DVE
            # copies (ctx rows 0..D-1 plus the denominator row D); the
            # denominators then take one merged off-critical-path chain:
            # DMA-hop both rows D to partition 0 of one [1,2,CH] tile, one
            # fast approximate reciprocal, one GpSimd partition-broadcast,
            # then the per-head multiplies.
            cs = []
            for hh in range(2):
                c_scr = cspool.tile([P, CH], f32, tag="cs",
                                    name=f"cs{qc}{hp}{hh}")
                if fast_tail and hh == 1:
                    nc.scalar.activation(c_scr[0 : D + 1, :],
                                         ctx_ps[hh][0 : D + 1, :], COPY)
                else:
                    nc.vector.tensor_copy(c_scr[0 : D + 1, :],
                                          ctx_ps[hh][0 : D + 1, :])
                cs.append(c_scr)
            lb = lbpool.tile([D, 2, CH], f32, tag="lb", name=f"lb{qc}{hp}")
            if fast_tail:
                # Last head pair: nothing else runs, so broadcast the
                # denominator rows on the otherwise-idle PE (ones-matmul)
                # instead of the long DMA-hop -> GpSimd-broadcast chain.
                for hh in range(2):
                    lb_ps = psumX.tile([P, CH], f32, tag="psX",
                                       name=f"lbps{qc}{hp}{hh}")
                    nc.tensor.matmul(lb_ps[0:D, :], ones_t[D : D + 1, :],
                                     cs[hh][D : D + 1, :],
                                     start=True, stop=True)
                    nc.vector.reciprocal_approx_fast(out=lb[:, hh, :],
                                                     in_=lb_ps[0:D, :])
            else:
                li = l0pool.tile([1, 2, CH], f32, tag="l0", name=f"l0{qc}{hp}")
                nc.gpsimd.dma_start(li[:, 0, :], cs[0][D : D + 1, :])
                nc.gpsimd.dma_start(li[:, 1, :], cs[1][D : D + 1, :])
                nc.vector.reciprocal_approx_fast(out=li[:], in_=li[:])
                nc.gpsimd.partition_broadcast(lb[:], li[:], channels=D)
            nc.vector.tensor_tensor(ctxT[0:D, col, :], cs[0][0:D, :],
                                    lb[:, 0, :], MUL)
            c2 = cs2pool.tile([P, CH], bf16, tag="cs2", name=f"cs2{qc}{hp}")
            nc.vector.tensor_tensor(c2[0:D, :], cs[1][0:D, :],
                                    lb[:, 1, :], MUL)
            nc.gpsimd.dma_start(ctxT[ds(64, 64), col, :], c2[0:D, :])

        # ---------- emission schedule ----------
        # prologue: V then K for kv [0:1024], Q for chunk A.  V first: its
        # inputs (wvt half + one 256KB x tile) are the fastest to land, so
        # the PE starts earliest.  The big resident-weight DMAs (wk, wq) go
        # on the GpSimd DMA queue, which is idle until attention starts, so
        # they don't head-of-line block the x-tile stream.
        # The first matmul's exact dependencies (xv0 + the two Wv quarters
        # for channel half 0) go on the GpSimd DMA queue: that engine boots
        # ~1.5us before the Sync engine and its queue is otherwise empty,
        # so the first-feed transfers start earlier and run in parallel
        # with the sync-queue stream.
        hk = KC // 2
        XV[0] = xvpool.tile([P, KC, P], bf16, tag="xv", name="xv0")
        nc.gpsimd.dma_start(XV[0][:], xkv_v[:, :, ts(0, P)])
        nc.gpsimd.dma_start(wvt[0][0][:], wv_v[:, 0:hk, 0:CH])
        nc.gpsimd.dma_start(wvt[0][1][:], wv_v[:, hk:KC, 0:CH])
        # The first partition_broadcast pays a ~6-9us one-time GpSimd IRAM
        # kernel load.  Issue a dummy one right after the critical DMAs
        # (so it can't delay them in the GpSimd stream), while the PE is
        # still waiting on the input feed.
        bc_warm = consts.tile([D, D], f32)
        nc.gpsimd.partition_broadcast(bc_warm[:], ones_t[0:1, :], channels=D)
        nc.sync.dma_start(bq_sb[:], bq)
        nc.sync.dma_start(bo_sb[:], bo)
        nc.sync.dma_start(wvt[1][0][:], wv_v[:, 0:hk, CH:C])
        nc.sync.dma_start(wvt[1][1][:], wv_v[:, hk:KC, CH:C])
        xv_fetch(1)
        wq_sb = wres.tile([P, KC, C], bf16, tag="wres", name="wq_r")
        wk_sb = wres.tile([P, KC, C], bf16, tag="wres", name="wk_r")
        # wk/wq are interleaved into the xv stream as 1MB halves, ordered by
        # when the PE consumes them (kt_proj reads wk columns 0:512 first
        # for m 0..3): no transfer delays one that is needed sooner.
        for i in range(8):
            v_proj(i)
            if i == 4:
                nc.sync.dma_start(wk_sb[:, :, 0:CH], wk_v[:, :, 0:CH])
            elif i == 6:
                nc.sync.dma_start(wk_sb[:, :, CH:C], wk_v[:, :, CH:C])
        kt_proj(wk_sb, 0, 0, 4)
        nc.sync.dma_start(wq_sb[:, :, 0:CH], wq_v[:, :, 0:CH])
        kt_proj(wk_sb, 0, 4, 8)
        nc.sync.dma_start(wq_sb[:, :, CH:C], wq_v[:, :, CH:C])
        kt_proj(wk_sb, 1, 0, 4); kt_proj(wk_sb, 1, 4, 8)
        q_proj(wq_sb, 0, 0, 4); q_proj(wq_sb, 0, 4, 8)

        mskA = mpool.tile([P, KC, CH], bf16, tag="mask", name="mA")
        nc.sync.dma_start(mskA[:], maskA_v)

        # chunk A attention, with kv[1024:2048] K/V projections and the
        # chunk-B Q projection interleaved as PE filler
        ctxT_A = ctxpool.tile([P, KC, CH], bf16, tag="ctxA", name="ctxA")
        fillers = ([lambda i=i: v_proj(i) for i in range(8, 16)]
                   + [lambda m0=m0: q_proj(wq_sb, 1, m0, m0 + 4)
                      for m0 in (0, 4)]
                   + [lambda: kt_proj(wk_sb, 2, 0, 1),
                      lambda: kt_proj(wk_sb, 3, 0, 1)])
        fi = 0
        for hp in range(NH // 2):
            attn_pair(0, hp, mskA, ctxT_A, hp)
            take = (len(fillers) - fi + (NH // 2 - hp) - 1) // (NH // 2 - hp)
            for _ in range(take):
                if fi < len(fillers):
                    fillers[fi](); fi += 1
        while fi < len(fillers):
            fillers[fi](); fi += 1

        # chunk-B mask prefetch reuses the xq slot (xq dead after q_proj(1));
        # wo reuses wq's slot (wq's last reader is the q_proj(1) filler).
        mskB = xqpool.tile([P, KC, CH], bf16, tag="xq", name="mB")
        nc.sync.dma_start(mskB[:], maskB_v)
        wo_sb = wres.tile([P, KC, C], bf16, tag="wres", name="wo_r")
        nc.sync.dma_start(wo_sb[:], wo_v)

        # chunk B attention, with chunk-A output projection interleaved.
        # ctxT_B is split: head pairs 0-6 write ctxT_Bm, pair 7 writes
        # ctxT_Bl.  o_proj(1)'s k=0..6 accumulation then only depends on
        # ctxT_Bm, so it prefills the PE while pair 7's softmax epilogue
        # chain drains; only the k=7 matmuls wait for the last tile.
        ctxT_Bm = ctxpool.tile([P, KC - 1, CH], bf16, tag="ctxBm", name="ctxBm")
        ctxT_Bl = ctxpool.tile([P, 1, CH], bf16, tag="ctxBl", name="ctxBl")
        rhs_A = lambda k: ctxT_A[:, k, :]
        rhs_B = lambda k: (ctxT_Bm[:, k, :] if k < KC - 1
                           else ctxT_Bl[:, 0, :])
        for hp in range(NH // 2):
            if hp < NH // 2 - 1:
                attn_pair(1, hp, mskB, ctxT_Bm, hp)
            else:
                attn_pair(1, hp, mskB, ctxT_Bl, 0, fast_tail=True)
            o_proj(wo_sb, 0, rhs_A, hp, dve_epi=True)
            if hp < NH // 2 - 1:
                # kt-hi for m=hp+1: lands well before pair hp+1 reaches
                # kvc>=8, and gives the exp-bound window PE work to chew on
                kt_proj(wk_sb, 2, hp + 1, hp + 2)
                kt_proj(wk_sb, 3, hp + 1, hp + 2)
        # m=2,3 borrow the psumX banks (free once the fast-tail broadcast
        # matmuls are consumed), so four k<7 accumulations can prefill the
        # PE while the last pair's epilogue chain drains.
        for m in range(KC):
            o_proj(wo_sb, 1, rhs_B, m,
                   pool=psumX if m in (2, 3) else None)

    nc.compile()
    return nc


def _shard_inputs(x, Wq, bq, bv, bo, WqT, WkT, WvT, WoT):
    """Build the 8 per-core input maps (bf16 data tensors, fp32 biases).

    bv is folded into the output-projection bias: ctx = ctx0 + 1*bv^T, so
    out = ctx0 @ Wo^T + (bo + Wo @ bv).  bk is dropped entirely: it adds
    q@bk to every score of a softmax row, which cancels in softmax."""
    bf = ml_dtypes.bfloat16
    in_maps = []
    rows = {0: (np.arange(0, 512), np.arange(1536, 2048)),
            1: (np.arange(512, 1024), np.arange(1024, 1536))}
    kv = np.arange(T)
    bq8 = np.ascontiguousarray((bq / 8.0).reshape(C // P, P).T)
    bo_f = bo + WoT.T @ bv
    bo8 = np.ascontiguousarray(bo_f.reshape(C // P, P).T)
    wq16, wk16 = WqT.astype(bf), WkT.astype(bf)
    wv16, wo16 = WvT.astype(bf), WoT.astype(bf)
    for b in range(B):
        xT = np.ascontiguousarray(x[b].T).astype(bf)     # (C, T)
        for h in range(2):
            qA, qB = rows[h]
            xqT = np.ascontiguousarray(xT[:, np.concatenate([qA, qB])])
            mA = (kv[:1024, None] <= qA[None, :]).astype(bf)
            mB = (kv[1024:, None] <= qB[None, :]).astype(bf)
            in_maps.append({
                "xqT": xqT, "xkvT": xT,
                "wqT": wq16, "wkT": wk16, "wvT": wv16, "woT": wo16,
                "bq": bq8, "bo": bo8,
                "maskA": np.ascontiguousarray(mA),
                "maskB": np.ascontiguousarray(mB),
            })
    return in_maps


def kernel(x, Wq, bq, Wk, bk, Wv, bv, Wo, bo):
    from concourse.bass_utils import run_bass_kernel_spmd

    x = np.asarray(x, np.float32)
    Wq = np.asarray(Wq, np.float32); bq = np.asarray(bq, np.float32)
    Wk = np.asarray(Wk, np.float32)
    Wv = np.asarray(Wv, np.float32); bv = np.asarray(bv, np.float32)
    Wo = np.asarray(Wo, np.float32); bo = np.asarray(bo, np.float32)

    if "nc" not in _CACHE:
        _CACHE["nc"] = _build()
    nc = _CACHE["nc"]

    WqT = np.ascontiguousarray(Wq.T / 8.0)
    WkT = np.ascontiguousarray(Wk.T)
    WvT = np.ascontiguousarray(Wv.T)
    WoT = np.ascontiguousarray(Wo.T)
    in_maps = _shard_inputs(x, Wq, bq, bv, bo, WqT, WkT, WvT, WoT)

    res = run_bass_kernel_spmd(nc, in_maps, core_ids=list(range(8)))
    outf = np.empty((B, T, C), np.float32)
    rows = {0: (np.arange(0, 512), np.arange(1536, 2048)),
            1: (np.arange(512, 1024), np.arange(1024, 1536))}
    for b in range(B):
        for h in range(2):
            o = res.results[2 * b + h]["out"]          # (C, 1024) transposed
            qA, qB = rows[h]
            outf[b, qA, :] = o[:, :512].T
            outf[b, qB, :] = o[:, 512:].T
    return outf
